# revision 22
# baseline (speedup 1.0000x reference)
"""3-layer GAT (8 heads x 32 hid, PyG GATConv semantics w/ self-loops) +
global mean pool + linear, distributed over 8 Trainium2 NeuronCores.

v2 design (bf16 data path, fused projection, overlapped collectives):

- Nodes partitioned into 8 contiguous ranges (dst-owner). Per layer each
  node has a 264-wide bf16 record [hp(256) | a_src(8)] (hp = h_in @ W);
  records are AllGathered in 4 row-chunks into lo/hi DRAM tables (row pad
  to 384 elems = 768B so dma_gather's 256B-granularity holds; int16 gather
  indices limit a table to 32768 rows, hence lo/hi).
- Edge phase: edges sorted by (half, dst_tile, src); per (dst_tile, half)
  run padded to whole 128-edge tiles; gather groups of KB=16 tiles span
  runs (one dma_gather each, amortizing Pool desc-gen). Per group: batched
  one-hot build (DVE, bf16); per tile: PE transpose -> alpha_dst matmul;
  e = lrelu(as+ad) (DVE), exp on Act engine straight into the rhs tile,
  rhs = h*ex via scalar_tensor_tensor (keeps DVE 2x mode with broadcast);
  segment-sum matmul accumulates [num|den] in PSUM; per run it is
  copied/added to an SBUF accumulator (lo/hi passes are separate sweeps).
- Epilogue per dst tile: out = num/den + bias; z = elu(out)+1 computed as
  max(h,0)+exp(min(h,0)); the -1 is folded into the next layer via a
  constant correction row (ones-column matmul) and into lin_b at the end.
  The next layer's projection zT @ Waug runs right here (2 bf16 transposes
  + 2 matmuls), so hh records stream out during the edge phase and the
  AllGather chunks for layer l+1 are dispatched mid-edge-phase, hidden
  behind compute. hh tables are double-buffered across layers.
- Final: per-graph partial sums via one-hot matmul, scatter to a [512,256]
  buffer, AllReduce, replicated final linear.

Self-contained: hardcodes problem shapes; host prep uses only graph
structure and parameter repacking.
"""
import numpy as np

import concourse.bass as bass
import concourse.bacc as bacc
import concourse.mybir as mybir
import concourse.tile as tile

P = 128
KB = 8                  # edge tiles per dma_gather group
HEADS, HID = 8, 32
DH = HEADS * HID        # 256
DA = DH + HEADS         # 264 = exh | ex (rhs/acc width)
REC = DH + 2 * HEADS    # 272 bf16 slots: hp(256 bf16) | as(8 f32)
ASF = DH // 2           # f32-view column where as starts (=128)
DW = DH + 2 * HEADS     # 272 = W | W@As | W@Adst
DG = 384                # gathered row elems (bf16): 768B, 256B-multiple
IN_CH = 128
NEG = 0.2
F32 = mybir.dt.float32
F32R = mybir.dt.float32r
F16 = mybir.dt.float16
BF16 = mybir.dt.bfloat16
I32 = mybir.dt.int32
I16 = mybir.dt.int16
TRP_BF16_PSUM = True    # transpose target dtype in PSUM


# ----------------------------------------------------------------- host prep

def _chunk_tiles(nt, nchunks):
    """Split nt tiles into nchunks contiguous runs (first gets remainder)."""
    base = nt // nchunks
    rem = nt - base * nchunks
    out = []
    for c in range(nchunks):
        out.append(base + (1 if c < rem else 0))
    return out


def host_prep(x, edge_index, batch, Ws, a_srcs, a_dsts, biases, lin_w, lin_b,
              N, E, G, ncores, nchunks=4):
    nl_real = N // ncores
    assert nl_real * ncores == N
    NL = ((nl_real + P - 1) // P) * P
    NT = NL // P
    GP = ((G + P - 1) // P) * P
    ctiles = _chunk_tiles(NT, nchunks)
    crows = [t * P for t in ctiles]                  # rows per core per chunk
    ct0 = np.cumsum([0] + ctiles)[:-1]               # first tile of chunk
    # global row base of each chunk block in the gathered tables
    cbase = np.cumsum([0] + [r * ncores for r in crows])[:-1]
    NG = NL * ncores
    # lo half = chunks [0, nchunks//2), hi = rest
    nlo = nchunks // 2
    HALF = int(cbase[nlo]) if nlo < nchunks else NG
    HI_SZ = NG - HALF
    assert HALF <= 32768 and HI_SZ <= 32768

    # map node id -> (gathered-table row, half)
    def gsrc_of(n):
        k = n // nl_real
        r = n % nl_real
        t = r // P
        c = np.searchsorted(np.asarray(ct0), t, side="right") - 1
        rr = r - ct0[c] * P
        return cbase[c] + k * crows[c] + rr

    src = np.concatenate([edge_index[0], np.arange(N, dtype=np.int64)])
    dst = np.concatenate([edge_index[1], np.arange(N, dtype=np.int64)])
    core_of = dst // nl_real
    dloc_all = dst - core_of * nl_real

    # vectorized gsrc
    k_a = src // nl_real
    r_a = src % nl_real
    t_a = r_a // P
    c_a = np.searchsorted(np.asarray(ct0), t_a, side="right") - 1
    gsrc_all = (np.asarray(cbase)[c_a] + k_a * np.asarray(crows)[c_a]
                + (r_a - np.asarray(ct0)[c_a] * P))
    half_all = (gsrc_all >= HALF).astype(np.int64)

    # per-core, per-(tile,half) edge lists sorted by (half, dst_local, src)
    cnt = np.zeros((ncores, NT, 2), np.int64)
    lists = [[[None, None] for _ in range(NT)] for _ in range(ncores)]
    for k in range(ncores):
        m = core_of == k
        g_k, d_k, h_k = gsrc_all[m], dloc_all[m], half_all[m]
        order = np.lexsort((g_k, h_k, d_k))
        g_k, d_k, h_k = g_k[order], d_k[order], h_k[order]
        t_k = d_k // P
        for t in range(NT):
            mt = t_k == t
            g_t, d_t, h_t = g_k[mt], d_k[mt] - t * P, h_k[mt]
            for half in (0, 1):
                mh = h_t == half
                g_h = g_t[mh] - HALF * half
                d_h = d_t[mh]
                if t == NT - 1 and half == 0 and NL > nl_real:
                    npad = NL - nl_real
                    g_h = np.concatenate([g_h, np.zeros(npad, np.int64)])
                    d_h = np.concatenate(
                        [d_h, np.arange(nl_real - t * P,
                                        nl_real - t * P + npad)])
                lists[k][t][half] = (g_h, d_h)
                cnt[k, t, half] = len(g_h)
    # uniform per-(tile,half) tile counts: max over cores, >= 1
    m_rt = np.maximum(np.ceil(cnt.max(axis=0) / P), 1).astype(np.int64)
    SM_half = [int(m_rt[:, h].sum()) for h in (0, 1)]
    SM = SM_half[0] + SM_half[1]

    # static schedule (same for every core): per half, the edge-tile list of
    # (tau, j_of_run, run_len); groups of KB tiles
    sched = {"m_rt": m_rt, "SM_half": SM_half, "SM": SM}

    in_maps = []
    batch = np.asarray(batch, np.int64)
    for k in range(ncores):
        dstrel = np.full((P, SM), 999.0, np.float32)
        idx16 = np.zeros((16, 8 * SM), np.int16)
        col = 0
        for half in (0, 1):
            for t in range(NT):
                g_h, d_h = lists[k][t][half]
                L = int(m_rt[t, half]) * P
                gi = np.zeros(L, np.int64)
                gi[:len(g_h)] = g_h
                dd = np.full(L, 999.0, np.float32)
                dd[:len(d_h)] = d_h
                ii = np.arange(L)
                dstrel[ii % P, col + ii // P] = dd
                idx16[:, col * 8:(col + int(m_rt[t, half])) * 8] = \
                    gi.reshape(-1, 16).T.astype(np.int16)
                col += int(m_rt[t, half])

        bl = batch[k * nl_real:(k + 1) * nl_real]
        g_base = int(bl[0])
        bshift = np.full((P, NT), 999.0, np.float32)
        bsv = (bl - g_base).astype(np.float32)
        for t in range(NT):
            seg = bsv[t * P:(t + 1) * P]
            bshift[:len(seg), t] = seg
        gslot = np.arange(g_base, g_base + P, dtype=np.int64)
        gslot = np.where(gslot < G, gslot, 99999).astype(np.int32)[:, None]

        xk = np.zeros((NL, IN_CH), np.float32)
        xk[:nl_real] = x[k * nl_real:(k + 1) * nl_real]
        xkT = np.ascontiguousarray(xk.T)             # [128, NL] f32 -> bf16

        im = {
            "xT": xkT,
            "idx16": np.tile(idx16, (8, 1)),
            "dstrel": _to_f16_bits(dstrel.reshape(-1)),
            "bshift": bshift.reshape(-1),
            "gslot": gslot,
        }
        in_maps.append(im)

    # replicated parameters (bf16, via uint16 bit views)
    def aug(W, a_s, a_d):
        As = np.zeros((DH, HEADS), np.float32)
        Ad = np.zeros((DH, HEADS), np.float32)
        for h in range(HEADS):
            As[h * HID:(h + 1) * HID, h] = a_s[h]
            Ad[h * HID:(h + 1) * HID, h] = a_d[h]
        return np.concatenate([W, W @ As, W @ Ad], axis=1).astype(np.float32)

    Waugs = [aug(Ws[l], a_srcs[l], a_dsts[l]) for l in range(3)]
    # elu(-1) correction rows for layer l+1 inputs: -sum_c Waug[c, :]
    corr = [(-Waugs[l].sum(axis=0, keepdims=True)).astype(np.float32)
            for l in range(3)]

    cnts = np.bincount(batch, minlength=G).astype(np.float32)
    invcnt = np.zeros((GP, 1), np.float32)
    invcnt[:G, 0] = 1.0 / np.maximum(cnts, 1.0)
    linb_adj = float(np.asarray(lin_b).reshape(-1)[0] - lin_w.sum())

    params = {
        "W0aug": Waugs[0],
        "W1aug": Waugs[1],
        "W2aug": Waugs[2],
        "corr1": corr[1],
        "corr2": corr[2],
        "b0": np.tile(biases[0][None, :], (P, 1)).astype(np.float32),
        "b1": np.tile(biases[1][None, :], (P, 1)).astype(np.float32),
        "b2": np.tile(biases[2][None, :], (P, 1)).astype(np.float32),
        "linw": np.tile(lin_w.reshape(1, DH), (P, 1)).astype(np.float32),
        "linb": np.full((P, 1), linb_adj, np.float32),
        "invcnt": invcnt,
    }
    for im in in_maps:
        im.update(params)

    cfg = dict(NL=NL, NT=NT, GP=GP, m_rt=tuple(map(tuple, m_rt)),
               SM_half=tuple(SM_half), SM=SM, ncores=ncores,
               ctiles=tuple(ctiles), HALF=HALF, HI_SZ=HI_SZ,
               nchunks=nchunks, nlo=nlo)
    return cfg, in_maps


def _to_f16_bits(a):
    """f32 array -> uint16 array holding the fp16 bit pattern."""
    return np.ascontiguousarray(a, np.float32).astype(
        np.float16).view(np.uint16)


# ------------------------------------------------------------- program build

def build_program(cfg, use_f32r=False, reps=1, dup=None, no_cc=False):
    del use_f32r
    NL, NT, GP = cfg["NL"], cfg["NT"], cfg["GP"]
    SM, ncores = cfg["SM"], cfg["ncores"]
    SM_half = cfg["SM_half"]
    m_rt = cfg["m_rt"]
    ctiles = list(cfg["ctiles"])
    nchunks, nlo = cfg["nchunks"], cfg["nlo"]
    HALF, HI_SZ = cfg["HALF"], cfg["HI_SZ"]
    NG = NL * ncores
    crows = [t * P for t in ctiles]
    ct0 = np.cumsum([0] + ctiles)[:-1]

    nc = bacc.Bacc("TRN2", target_bir_lowering=False, debug=False,
                   num_devices=ncores, dynamic_dma_scratch_size=32768)
    # ---------------- I/O (bf16 params arrive as uint16 bit patterns)
    xT_in = nc.dram_tensor("xT", [P, NL], F32, kind="ExternalInput")
    idx16 = nc.dram_tensor("idx16", [P, 8 * SM], I16, kind="ExternalInput")
    dstrel = nc.dram_tensor("dstrel", [P * SM], mybir.dt.uint16,
                            kind="ExternalInput")
    bshift = nc.dram_tensor("bshift", [P * NT], F32, kind="ExternalInput")
    gslot = nc.dram_tensor("gslot", [P, 1], I32, kind="ExternalInput")
    Waug = [nc.dram_tensor(f"W{l}aug", [IN_CH if l == 0 else DH, DW],
                           F32, kind="ExternalInput")
            for l in range(3)]
    corr_d = [None,
              nc.dram_tensor("corr1", [1, DW], F32, kind="ExternalInput"),
              nc.dram_tensor("corr2", [1, DW], F32, kind="ExternalInput")]
    bias = [nc.dram_tensor(f"b{l}", [P, DH], F32, kind="ExternalInput")
            for l in range(3)]
    linw = nc.dram_tensor("linw", [P, DH], F32, kind="ExternalInput")
    linb = nc.dram_tensor("linb", [P, 1], F32, kind="ExternalInput")
    invcnt = nc.dram_tensor("invcnt", [GP, 1], F32, kind="ExternalInput")
    y = nc.dram_tensor("y", [GP, 1], F32, kind="ExternalOutput")

    # static edge-tile schedule: per half, list of (tau, first, last)
    sched = []
    for half in (0, 1):
        tiles = []
        for t in range(NT):
            m = m_rt[t][half]
            for j in range(m):
                tiles.append((t, j == 0, j == m - 1))
        sched.append(tiles)
    assert len(sched[0]) == SM_half[0] and len(sched[1]) == SM_half[1]
    # chunk of a dst tile (for AG dispatch positions)
    chunk_of_tile = np.searchsorted(np.asarray(ct0), np.arange(NT),
                                    side="right") - 1
    # last dst tile per chunk
    chunk_last_tile = [int(ct0[c] + ctiles[c] - 1) for c in range(nchunks)]

    with tile.TileContext(nc) as tc:
        with tc.tile_pool(name="const", bufs=1) as cst, \
             tc.tile_pool(name="dram", bufs=1, space="DRAM") as dram, \
             tc.tile_pool(name="wk", bufs=4) as wk, \
             tc.tile_pool(name="epool", bufs=3) as ep, \
             tc.tile_pool(name="gpool", bufs=2) as gp, \
             tc.tile_pool(name="opool", bufs=2) as op, \
             tc.tile_pool(name="accsb", bufs=1) as accp, \
             tc.tile_pool(name="psAcc", bufs=2, space="PSUM") as psAcc, \
             tc.tile_pool(name="psAdg", bufs=1, space="PSUM") as psAdg, \
             tc.tile_pool(name="psTrp", bufs=1, space="PSUM") as psTrp, \
             tc.tile_pool(name="psPrj", bufs=2, space="PSUM") as psPrj, \
             tc.tile_pool(name="psPool", bufs=1, space="PSUM") as psPool:

            # ---------------- DRAM intermediates (double-buffered tables)
            hh_lo = [dram.tile([HALF, DG], F16, name=f"hh_lo{i}")
                     for i in range(2)]
            hh_hi = [dram.tile([HI_SZ, DG], F16, name=f"hh_hi{i}")
                     for i in range(2)]
            # hh_local rows are full DG wide (pad cols zeroed once) so the
            # AllGather ships contiguous rows
            hh_local = [dram.tile([NL, DG], F16, name=f"hh_local{i}")
                        for i in range(2)]
            pool_loc = dram.tile([GP, DH], F32)
            pool_sum = dram.tile([GP, DH], F32)

            # ---------------- constants
            iota_i = cst.tile([P, P], I32)
            nc.gpsimd.iota(iota_i[:], pattern=[[1, P]], base=0,
                           channel_multiplier=0)
            iota_f = cst.tile([P, P], F32)
            nc.vector.tensor_copy(iota_f[:], iota_i[:])

            iota_ci = cst.tile([P, 1], I32)
            nc.gpsimd.iota(iota_ci[:], pattern=[[0, 1]], base=0,
                           channel_multiplier=1)
            iota_cf = cst.tile([P, 1], F32)
            nc.vector.tensor_copy(iota_cf[:], iota_ci[:])
            ident_b = cst.tile([P, P], F16)
            nc.vector.tensor_tensor(out=ident_b[:],
                                    in0=iota_cf[:, 0:1].to_broadcast([P, P]),
                                    in1=iota_f[:],
                                    op=mybir.AluOpType.is_equal)
            ident_r = cst.tile([P, P], F32R)
            nc.vector.tensor_tensor(out=ident_r[:],
                                    in0=iota_cf[:, 0:1].to_broadcast([P, P]),
                                    in1=iota_f[:],
                                    op=mybir.AluOpType.is_equal)
            ones_f = cst.tile([1, P], F32)
            nc.gpsimd.memset(ones_f[:], 1.0)
            ones_row = cst.tile([1, P], F32R)
            nc.vector.tensor_copy(ones_row[:], ones_f[:])
            # iota tiled KB times along the free dim: value = j mod P
            iota_ti = cst.tile([P, KB * P], I16)
            nc.gpsimd.iota(iota_ti[:].rearrange("p (q j) -> p q j", q=KB),
                           pattern=[[0, KB], [1, P]], base=0,
                           channel_multiplier=0)
            iota_tb = cst.tile([P, KB * P], F16)
            nc.vector.tensor_copy(iota_tb[:], iota_ti[:])

            idx_all = cst.tile([P, 8 * SM], I16)
            nc.sync.dma_start(idx_all[:], idx16[:, :])
            dst_all = cst.tile([P, SM], F16)
            nc.sync.dma_start(
                dst_all[:].bitcast(mybir.dt.uint16),
                dstrel[:].rearrange("(p j) -> p j", j=SM))
            bsh_all = cst.tile([P, NT], F32)
            nc.sync.dma_start(bsh_all[:], bshift[:].rearrange(
                "(p j) -> p j", j=NT))

            W_t = []
            for l in range(3):
                cin = IN_CH if l == 0 else DH
                tiles = []
                for kk in range(cin // P):
                    t = cst.tile([P, DW], F32R, tag=f"W{l}_{kk}",
                                 name=f"W{l}_{kk}")
                    nc.gpsimd.dma_start(t[:], Waug[l][kk * P:(kk + 1) * P, :])
                    tiles.append(t)
                W_t.append(tiles)
            W0f = cst.tile([P, DW], F32, name="W0f")
            nc.sync.dma_start(W0f[:], Waug[0][:, :])
            corr_t = [None, None, None]
            for l in (1, 2):
                t = cst.tile([1, DW], F32R, tag=f"corr{l}", name=f"corr{l}")
                nc.gpsimd.dma_start(t[:], corr_d[l][:, :])
                corr_t[l] = t
            bias_t = []
            for l in range(3):
                t = cst.tile([P, DH], F32, tag=f"bias{l}", name=f"bias{l}")
                nc.sync.dma_start(t[:], bias[l][:, :])
                bias_t.append(t)
            linw_t = cst.tile([P, DH], F32)
            nc.sync.dma_start(linw_t[:], linw[:, :])
            linb_t = cst.tile([P, 1], F32)
            nc.sync.dma_start(linb_t[:], linb[:, :])
            gslot_t = cst.tile([P, 1], I32)
            nc.sync.dma_start(gslot_t[:], gslot[:, :])

            # persistent SBUF: per-layer ad [P, NT*8] and accumulators
            ad_sb = [cst.tile([P, NT * HEADS], F32, name=f"ad_sb{i}")
                     for i in range(2)]
            acc_sb = accp.tile([P, NT * DA], F32, name="acc_sb")

            # zero the pad columns of hh_local once (records write 0:DA; the
            # AllGather ships full DG rows so table pads come from here)
            zpad = cst.tile([P, DG - REC], F16)
            nc.gpsimd.memset(zpad[:], 0.0)
            for i in range(2):
                for t in range(NT):
                    nc.sync.dma_start(
                        hh_local[i][t * P:(t + 1) * P, REC:DG], zpad[:])

            # ---------------- helpers
            def all_gather_chunk(l, c):
                """AG chunk c of layer l's records into table parity l%2."""
                par = l % 2
                r0 = int(ct0[c]) * P
                rows = crows[c]
                in_ap = hh_local[par][r0:r0 + rows, :]
                gbase = int(np.cumsum([0] + [r * ncores for r in crows])[c])
                if c < nlo:
                    out_t, off = hh_lo[par], gbase
                else:
                    out_t, off = hh_hi[par], gbase - HALF
                out_ap = out_t[off:off + rows * ncores, :]
                if no_cc:
                    nc.sync.dma_start(out_t[off:off + rows, :], in_ap)
                    return
                nc.gpsimd.collective_compute(
                    "AllGather", mybir.AluOpType.bypass,
                    ins=[in_ap.opt()], outs=[out_ap.opt()],
                    replica_groups=[list(range(ncores))])

            def store_records(l, t, src_ps):
                """PSUM [P, DW] f32 -> hh_local rows (hp bf16 | as f32)
                + ad_sb (f32)."""
                par = l % 2
                hp_sb = ep.tile([P, REC], F16, tag="hp_sb")
                nc.scalar.activation(hp_sb[:, 0:DH], src_ps[:, 0:DH],
                                     mybir.ActivationFunctionType.Copy)
                nc.vector.tensor_copy(
                    hp_sb[:, DH:REC].bitcast(F32), src_ps[:, DH:DH + HEADS])
                nc.vector.tensor_copy(
                    ad_sb[par][:, t * HEADS:(t + 1) * HEADS],
                    src_ps[:, DH + HEADS:DW])
                nc.sync.dma_start(hh_local[par][t * P:(t + 1) * P, 0:REC],
                                  hp_sb[:])

            def node_phase0():
                """hp0 = x @ W0aug via pre-transposed x."""
                for t in range(NT):
                    xt = wk.tile([P, P], F32, tag="xt")
                    nc.sync.dma_start(xt[:], xT_in[:, t * P:(t + 1) * P])
                    ps = psPrj.tile([P, DW], F32, space="PSUM", tag="prj")
                    nc.tensor.matmul(ps[:], lhsT=xt[:],
                                     rhs=W0f[:], start=True, stop=True)
                    store_records(0, t, ps)
                for c in range(nchunks):
                    all_gather_chunk(0, c)

            def epilogue(l, t, pool_state):
                """num/den -> elu+1 -> (proj to layer l+1) or pool."""
                acc = acc_sb[:, t * DA:(t + 1) * DA]
                inv = ep.tile([P, HEADS], F32, tag="inv")
                nc.vector.reciprocal(inv[:], acc[:, DH:DA])
                h0 = ep.tile([P, DH], F32, tag="h0")
                nc.vector.scalar_tensor_tensor(
                    out=h0[:].rearrange("p (h c) -> p h c", h=HEADS),
                    in0=acc[:, 0:DH].rearrange("p (h c) -> p h c", h=HEADS),
                    scalar=1.0, op0=mybir.AluOpType.mult,
                    in1=inv[:, :, None].to_broadcast([P, HEADS, HID]),
                    op1=mybir.AluOpType.mult)
                nc.vector.scalar_tensor_tensor(
                    out=h0[:], in0=h0[:], scalar=1.0,
                    op0=mybir.AluOpType.mult, in1=bias_t[l][:],
                    op1=mybir.AluOpType.add)
                tm = ep.tile([P, DH], F32, tag="tm")
                nc.vector.tensor_scalar_min(tm[:], h0[:], 0.0)
                nc.scalar.activation(tm[:], tm[:],
                                     mybir.ActivationFunctionType.Exp)
                z = ep.tile([P, DH], F32R, tag="z")
                nc.vector.scalar_tensor_tensor(
                    out=z[:], in0=h0[:], scalar=0.0,
                    op0=mybir.AluOpType.max, in1=tm[:],
                    op1=mybir.AluOpType.add)
                if l < 2:
                    # fused projection to layer l+1
                    ps = psPrj.tile([P, DW], F32, space="PSUM", tag="prj")
                    ztr = psTrp.tile([P, 2 * P], F32R,
                                     space="PSUM", tag="ztr", name="ztr")
                    for kk in range(2):
                        nc.tensor.transpose(
                            out=ztr[:, kk * P:(kk + 1) * P],
                            in_=z[:, kk * P:(kk + 1) * P],
                            identity=ident_r[:])
                    zT = ep.tile([P, 2 * P], F32R, tag="zT")
                    nc.vector.tensor_copy(zT[:], ztr[:])
                    for kk in range(2):
                        nc.tensor.matmul(ps[:], lhsT=zT[:, kk * P:(kk + 1) * P],
                                         rhs=W_t[l + 1][kk][:],
                                         start=(kk == 0), stop=False)
                    nc.tensor.matmul(
                        ps[:], lhsT=ones_row[0:1, :],
                        rhs=corr_t[l + 1][0:1, :], start=False, stop=True)
                    store_records(l + 1, t, ps)
                else:
                    ohp = ep.tile([P, P], F32R, tag="ohp")
                    nc.vector.scalar_tensor_tensor(
                        out=ohp[:],
                        in0=bsh_all[:, t:t + 1].to_broadcast([P, P]),
                        scalar=1.0, op0=mybir.AluOpType.mult,
                        in1=iota_f[:], op1=mybir.AluOpType.is_equal)
                    nc.tensor.matmul(pool_state[:], lhsT=ohp[:], rhs=z[:],
                                     start=(t == 0), stop=(t == NT - 1))

            def edge_phase(l, ag_next):
                """One full edge sweep: lo pass then hi pass."""
                par = l % 2
                pool_state = None
                if l == 2:
                    pool_state = psPool.tile([P, DH], F32, space="PSUM",
                                             tag="pool", name="pool_ps")
                # AG dispatch positions: chunk c emitted in hi pass after the
                # group whose last tile's tau passed chunk_last_tile[c] + 1
                ag_emitted = [False] * nchunks

                col = 0                      # global tile column (idx/dst)
                for half in (0, 1):
                    tiles = sched[half]
                    src_t = (hh_lo, hh_hi)[half][par]
                    ntile = len(tiles)
                    g0 = 0
                    while g0 < ntile:
                        gk = min(KB, ntile - g0)
                        cols = col + g0
                        # ---- gather group
                        g4 = gp.tile([P, KB * DG], F16, tag="hhg")
                        nc.gpsimd.dma_gather(
                            out_ap=g4[:, 0:gk * DG].rearrange(
                                "p (q d) -> p q d", q=gk),
                            in_ap=src_t[:, :],
                            idxs_ap=idx_all[:, cols * 8:(cols + gk) * 8],
                            num_idxs=gk * P, num_idxs_reg=gk * P,
                            elem_size=DG)
                        # ---- batched one-hot build (bf16, keeps 2x)
                        ohb = op.tile([P, KB * P], F16, tag="ohb")
                        nc.vector.scalar_tensor_tensor(
                            out=ohb[:, 0:gk * P].rearrange(
                                "p (q j) -> p q j", q=gk),
                            in0=dst_all[:, cols:cols + gk][:, :, None]
                                .to_broadcast([P, gk, P]),
                            scalar=1.0, op0=mybir.AluOpType.mult,
                            in1=iota_tb[:, 0:gk * P].rearrange(
                                "p (q j) -> p q j", q=gk),
                            op1=mybir.AluOpType.is_equal)
                        # ---- per-tile transpose + alpha_dst matmul
                        adg = psAdg.tile([P, KB * HEADS], F32, space="PSUM",
                                         tag="adg")
                        for qq in range(0, gk, 4):
                            qe = min(qq + 4, gk)
                            nb = qe - qq
                            trp = psTrp.tile(
                                [P, 4 * P], F16 if TRP_BF16_PSUM else F32,
                                space="PSUM", tag="trp")
                            for q in range(qq, qe):
                                nc.tensor.transpose(
                                    out=trp[:, (q - qq) * P:(q - qq + 1) * P],
                                    in_=ohb[:, q * P:(q + 1) * P],
                                    identity=ident_b[:])
                            ohT = ep.tile([P, 4 * P], F32, tag="ohT")
                            nc.scalar.activation(
                                ohT[:, 0:nb * P], trp[:, 0:nb * P],
                                mybir.ActivationFunctionType.Copy)
                            for q in range(qq, qe):
                                tau = tiles[g0 + q][0]
                                nc.tensor.matmul(
                                    adg[:, q * HEADS:(q + 1) * HEADS],
                                    lhsT=ohT[:, (q - qq) * P:(q - qq + 1) * P],
                                    rhs=ad_sb[par][
                                        :, tau * HEADS:(tau + 1) * HEADS],
                                    start=True, stop=True)
                        # ---- e = lrelu(as + ad); ex -> rhs cols
                        e4 = ep.tile([P, KB * HEADS], F32, tag="e4")
                        gf = g4[:].bitcast(F32)
                        nc.vector.tensor_add(
                            e4[:, 0:gk * HEADS].rearrange(
                                "p (q h) -> p q h", q=gk),
                            gf[:, 0:gk * (DG // 2)].rearrange(
                                "p (q d) -> p q d", q=gk)[
                                :, :, ASF:ASF + HEADS],
                            adg[:, 0:gk * HEADS].rearrange(
                                "p (q h) -> p q h", q=gk))
                        nc.vector.scalar_tensor_tensor(
                            out=e4[:, 0:gk * HEADS], in0=e4[:, 0:gk * HEADS],
                            scalar=NEG, in1=e4[:, 0:gk * HEADS],
                            op0=mybir.AluOpType.mult,
                            op1=mybir.AluOpType.max)
                        rhs4 = op.tile([P, KB * DA], F16, tag="rhs")
                        nc.scalar.activation(
                            rhs4[:, 0:gk * DA].rearrange(
                                "p (q d) -> p q d", q=gk)[:, :, DH:DA],
                            e4[:, 0:gk * HEADS].rearrange(
                                "p (q h) -> p q h", q=gk),
                            mybir.ActivationFunctionType.Exp)
                        # rhs[:, :256] = h * ex (4-dim bcast only valid on
                        # InstTensorTensor per the BIR verifier)
                        nc.vector.tensor_mul(
                            rhs4[:, 0:gk * DA].rearrange(
                                "p (q d) -> p q d", q=gk)[:, :, 0:DH]
                                .rearrange("p q (h c) -> p q h c", h=HEADS),
                            g4[:, 0:gk * DG].rearrange(
                                "p (q d) -> p q d", q=gk)[:, :, 0:DH]
                                .rearrange("p q (h c) -> p q h c", h=HEADS),
                            rhs4[:, 0:gk * DA].rearrange(
                                "p (q d) -> p q d", q=gk)[:, :, DH:DA][
                                :, :, :, None].to_broadcast(
                                [P, gk, HEADS, HID]))
                        # ---- segment-sum matmuls + run boundaries
                        for q in range(gk):
                            tau, first, last_t = tiles[g0 + q]
                            if first:
                                acc_ps = psAcc.tile([P, DA], F32,
                                                    space="PSUM", tag="acc")
                                cur_acc[0] = acc_ps
                            nc.tensor.matmul(
                                cur_acc[0][:],
                                lhsT=ohb[:, q * P:(q + 1) * P],
                                rhs=rhs4[:, q * DA:(q + 1) * DA],
                                start=first, stop=last_t)
                            if last_t:
                                sb = acc_sb[:, tau * DA:(tau + 1) * DA]
                                if half == 0:
                                    nc.scalar.activation(
                                        sb, cur_acc[0][:],
                                        mybir.ActivationFunctionType.Copy)
                                else:
                                    nc.vector.tensor_add(sb, sb,
                                                         cur_acc[0][:])
                                    epilogue(l, tau, pool_state)
                                    if ag_next and half == 1:
                                        for c in range(nchunks):
                                            if (not ag_emitted[c]
                                                    and tau > chunk_last_tile[c]):
                                                all_gather_chunk(l + 1, c)
                                                ag_emitted[c] = True
                        g0 += gk
                    col += len(tiles)
                if ag_next:
                    for c in range(nchunks):
                        if not ag_emitted[c]:
                            all_gather_chunk(l + 1, c)
                            ag_emitted[c] = True
                return pool_state

            cur_acc = [None]

            # ---------------- run
            for _rep in range(reps):
                node_phase0()
                if dup == "node":
                    node_phase0()
                for l in range(3):
                    if dup == "ag" and l > 0:
                        for c in range(nchunks):
                            all_gather_chunk(l, c)
                    pool_state = edge_phase(l, ag_next=(l < 2))
                    if dup == "edge":
                        edge_phase(l, ag_next=False)

                # ---- pool scatter + AllReduce
                zt = ep.tile([P, DH], F32, tag="zero")
                nc.gpsimd.memset(zt[:], 0.0)
                for b in range(GP // P):
                    nc.sync.dma_start(pool_loc[b * P:(b + 1) * P, :], zt[:])
                pl = ep.tile([P, DH], F32, tag="plocal")
                nc.vector.tensor_copy(pl[:], pool_state[:])
                nc.gpsimd.indirect_dma_start(
                    out=pool_loc[:, :],
                    out_offset=bass.IndirectOffsetOnAxis(
                        ap=gslot_t[:, 0:1], axis=0),
                    in_=pl[:, :], in_offset=None,
                    bounds_check=GP - 1, oob_is_err=False)
                if no_cc:
                    nc.sync.dma_start(pool_sum[:, :], pool_loc[:, :])
                else:
                    nc.gpsimd.collective_compute(
                        "AllReduce", mybir.AluOpType.add,
                        ins=[pool_loc[:, :].opt()],
                        outs=[pool_sum[:, :].opt()],
                        replica_groups=[list(range(ncores))])

            # ---- final linear: y = (pool_sum * invcnt) @ lin_w + lin_b'
            for b in range(GP // P):
                pt = ep.tile([P, DH], F32, tag="psum_t")
                nc.sync.dma_start(pt[:], pool_sum[b * P:(b + 1) * P, :])
                ic = ep.tile([P, 1], F32, tag="ic")
                nc.sync.dma_start(ic[:], invcnt[b * P:(b + 1) * P, :])
                mulw = ep.tile([P, DH], F32, tag="mulw")
                nc.vector.tensor_mul(mulw[:], pt[:], linw_t[:])
                rs = ep.tile([P, 1], F32, tag="rs")
                nc.vector.reduce_sum(rs[:], mulw[:], axis=mybir.AxisListType.X)
                nc.vector.tensor_mul(rs[:], rs[:], ic[:])
                nc.vector.tensor_add(rs[:], rs[:], linb_t[:])
                nc.sync.dma_start(y[b * P:(b + 1) * P, :], rs[:])

    nc.compile()
    return nc


# ------------------------------------------------------------------- runner

class SpmdRunner:
    def __init__(self, nc, n_cores):
        import jax
        from jax.sharding import Mesh, PartitionSpec
        from jax.experimental.shard_map import shard_map
        from concourse.bass2jax import (
            _bass_exec_p, install_neuronx_cc_hook, partition_id_tensor)
        self.jax = jax
        install_neuronx_cc_hook()
        self.nc = nc
        self.n_cores = n_cores
        partition_name = (nc.partition_id_tensor.name
                          if nc.partition_id_tensor else None)
        in_names, out_names, out_avals, zero_outs = [], [], [], []
        for alloc in nc.m.functions[0].allocations:
            if not isinstance(alloc, mybir.MemoryLocationSet):
                continue
            name = alloc.memorylocations[0].name
            if alloc.kind == "ExternalInput":
                if name != partition_name and name != (
                        nc.dbg_addr.name if nc.dbg_addr else None):
                    in_names.append(name)
            elif alloc.kind == "ExternalOutput":
                out_names.append(name)
                shape = tuple(alloc.tensor_shape)
                dtype = mybir.dt.np(alloc.dtype)
                out_avals.append(jax.core.ShapedArray(shape, dtype))
                zero_outs.append(np.zeros(shape, dtype))
        self.in_names, self.out_names = in_names, out_names
        self.out_avals, self.zero_outs = out_avals, zero_outs
        n_params = len(in_names)
        all_in_names = list(in_names) + list(out_names)
        has_dbg = nc.dbg_addr is not None
        if has_dbg:
            all_in_names.append(nc.dbg_addr.name)
        if partition_name is not None:
            all_in_names.append(partition_name)

        def _body(*args):
            operands = list(args)
            if has_dbg:
                operands.append(jax.numpy.zeros((1, 2), jax.numpy.uint32))
            if partition_name is not None:
                operands.append(partition_id_tensor())
            outs = _bass_exec_p.bind(
                *operands, out_avals=tuple(out_avals),
                in_names=tuple(all_in_names), out_names=tuple(out_names),
                lowering_input_output_aliases=(),
                sim_require_finite=False, sim_require_nnan=False, nc=nc)
            return tuple(outs)

        devices = jax.devices()[:n_cores]
        assert len(devices) == n_cores
        mesh = Mesh(np.asarray(devices), ("core",))
        in_specs = (PartitionSpec("core"),) * (n_params + len(out_names))
        out_specs = (PartitionSpec("core"),) * len(out_names)
        self.fn = jax.jit(
            shard_map(_body, mesh=mesh, in_specs=in_specs,
                      out_specs=out_specs, check_rep=False),
            keep_unused=True)

    def prepare(self, in_maps):
        per_core = [[np.ascontiguousarray(m[nm]) for nm in self.in_names]
                    for m in in_maps]
        concat_in = [
            np.concatenate([per_core[c][i] for c in range(self.n_cores)],
                           axis=0)
            for i in range(len(self.in_names))]
        concat_zero = [
            np.zeros((self.n_cores * z.shape[0], *z.shape[1:]), z.dtype)
            for z in self.zero_outs]
        args = [self.jax.device_put(a) for a in concat_in + concat_zero]
        for a in args:
            a.block_until_ready()
        return args

    def run(self, args):
        outs = self.fn(*args)
        self.jax.block_until_ready(outs)
        return outs

    def results(self, outs):
        res = []
        for c in range(self.n_cores):
            m = {}
            for i, nm in enumerate(self.out_names):
                m[nm] = np.asarray(outs[i]).reshape(
                    self.n_cores, *self.out_avals[i].shape)[c]
            res.append(m)
        return res


# -------------------------------------------------------------------- kernel

_CACHE = {}

N_FULL, E_FULL, G_FULL, NCORES = 50000, 800000, 512, 8
USE_F32R = False


def kernel(x, edge_index, batch,
           W0, a_src0, a_dst0, bias0,
           W1, a_src1, a_dst1, bias1,
           W2, a_src2, a_dst2, bias2,
           lin_w, lin_b):
    x = np.asarray(x, np.float32)
    edge_index = np.asarray(edge_index, np.int64)
    batch = np.asarray(batch, np.int64)
    N, E, G = x.shape[0], edge_index.shape[1], G_FULL

    cfg, in_maps = host_prep(
        x, edge_index, batch,
        [np.asarray(W0, np.float32), np.asarray(W1, np.float32),
         np.asarray(W2, np.float32)],
        [np.asarray(a_src0, np.float32), np.asarray(a_src1, np.float32),
         np.asarray(a_src2, np.float32)],
        [np.asarray(a_dst0, np.float32), np.asarray(a_dst1, np.float32),
         np.asarray(a_dst2, np.float32)],
        [np.asarray(bias0, np.float32), np.asarray(bias1, np.float32),
         np.asarray(bias2, np.float32)],
        np.asarray(lin_w, np.float32), np.asarray(lin_b, np.float32),
        N, E, G, NCORES)

    key = (cfg["NL"], cfg["NT"], cfg["GP"], cfg["m_rt"], cfg["SM"],
           cfg["ncores"])
    if key not in _CACHE:
        nc = build_program(cfg)
        _CACHE[key] = (nc, SpmdRunner(nc, NCORES))
    nc, runner = _CACHE[key]

    args = runner.prepare(in_maps)
    outs = runner.run(args)
    res = runner.results(outs)
    return res[0]["y"][:G].astype(np.float32)
